# revision 1
# baseline (speedup 1.0000x reference)
"""Trainium2 Bass kernel for 3x3 VALID conv: x[32,128,64,64] * w[256,128,3,3] + bias.

Strategy:
  - Data-parallel over batch: 8 cores x 4 images each; weights/bias replicated.
  - Per core: implicit GEMM. Contraction dim = C_IN = 128 = partition dim.
    For each filter tap (u,v), accumulate
        psum[o, i, j] += W[c, o; u,v].T @ x[c, i+u, j+v]
    with the moving operand a strided [r, 62] view of a [C, rows, W] input
    piece, so only the 62 valid output columns are streamed.
  - bf16 x/w/y (accuracy ~4e-3 << 2e-2 tol): halves DMA bytes and SBUF
    bandwidth; matmul still 1 cycle/row. PSUM accumulates fp32.
  - Critical-path fusion: the host packs [x image0 rows 0-9 | half0 taps |
    half1 taps 6-8] into one DRAM tensor loaded by ONE DMA as the Sync
    ring's first batch (~10.7us); half1 taps 0-5 (needed ~2.5us later) are
    the Scalar ring's first batch. Tile dependencies are tile-granular and
    a ring's 2nd DMA batch lands ~3.5us late, so only first batches carry
    critical data.
  - Dummy matmuls on a zeroed scratch tile ramp the PE DVFS p-state until
    that DMA lands; a stream gap would reset the ramp to half clock.
  - Rings: fused xw + y stores on Sync (fast queue class); x piece prefetch
    and bias on Scalar (latency-tolerant).
  - The very last PSUM group is split in two so the first sub-group's
    evacuation + store hide under the second's matmuls (shorter tail).
"""

import numpy as np
import ml_dtypes

import concourse.bacc as bacc
import concourse.tile as tile
from concourse import mybir
from concourse.bass_utils import run_bass_kernel_spmd

N_CORES = 8
B_FULL, C_IN, H, W = 32, 128, 64, 64
C_OUT, KH, KW = 256, 3, 3
B_LOC = B_FULL // N_CORES          # images per core
H_OUT = W_OUT = H - KH + 1         # 62
N_HALF = C_OUT // 128              # 2 output-channel halves
RPC = 8                            # output rows per PSUM chunk (one bank)
N_CHUNKS = (H_OUT + RPC - 1) // RPC
P_ROWS = 2 * RPC + KH - 1          # input rows per 2-chunk x piece (18)
N_PIECES = 4                       # pieces [0:18],[16:34],[32:50],[48:64]
N_WARM = 8                         # DVFS warm-up matmuls
X0_ROWS = RPC + KH - 1             # fused tile: x image0 rows 0-9 ...
X0_LEN = X0_ROWS * W
W_LEN = N_HALF * KH * KW * 128
H1A = 6                            # half1 taps 0-5 ride the Scalar ring
WF_LEN = (KH * KW + KH * KW - H1A) * 128   # ... h0 taps + h1 taps 6-8
XW_LEN = X0_LEN + WF_LEN

_cached = {}


def _build_nc():
    f32 = mybir.dt.float32
    bf16 = mybir.dt.bfloat16
    nc = bacc.Bacc()

    xw_d = nc.declare_dram_parameter("xw0", [C_IN, XW_LEN], bf16, isOutput=False)
    w1a_d = nc.declare_dram_parameter("w1a", [C_IN, H1A, 128], bf16, isOutput=False)
    x_d = nc.declare_dram_parameter("x", [B_LOC, C_IN, H, W], bf16, isOutput=False)
    b_d = nc.declare_dram_parameter("bias_in", [128, N_HALF], f32, isOutput=False)
    y_d = nc.declare_dram_parameter(
        "y", [B_LOC, N_HALF, 128, H_OUT, W_OUT], bf16, isOutput=True
    )

    with tile.TileContext(nc) as tc:
        with (
            tc.tile_pool(name="const", bufs=1) as cpool,
            tc.tile_pool(name="xin", bufs=5) as xpool,
            tc.tile_pool(name="out", bufs=4) as opool,
            tc.tile_pool(name="psum", bufs=4, space="PSUM") as ppool,
            tc.tile_pool(name="warm", bufs=1, space="PSUM") as wpool,
        ):
            ct = cpool.tile([C_IN, XW_LEN], bf16)
            w1a_t = cpool.tile([C_IN, H1A, 128], bf16)
            b_t = cpool.tile([128, N_HALF], f32)
            scr = cpool.tile([128, 512], bf16)

            nc.vector.memset(scr[:], 0.0)

            # Critical-path DMAs: the slim fused tile (x piece0 + h0 taps +
            # h1 taps 6-8) is the Sync ring's first batch; h1 taps 0-5 are
            # the Scalar ring's first batch (needed ~2.5us after stream
            # start, and a ring's FIRST batch lands early).
            nc.sync.dma_start(ct[:], xw_d[:])
            nc.scalar.dma_start(w1a_t[:], w1a_d[:])
            nc.scalar.dma_start(b_t[:], b_d[:])

            x0v = ct[:, 0:X0_LEN].rearrange("c (h w) -> c h w", w=W)
            wv0 = ct[:, X0_LEN : X0_LEN + KH * KW * 128].rearrange(
                "c (t o) -> c t o", t=KH * KW
            )
            wv1b = ct[:, X0_LEN + KH * KW * 128 : XW_LEN].rearrange(
                "c (t o) -> c t o", t=KH * KW - H1A
            )

            def lhsT(half, uv):
                if half == 0:
                    return wv0[:, uv, :]
                return w1a_t[:, uv, :] if uv < H1A else wv1b[:, uv - H1A, :]

            # Ramp the PE p-state while the fused DMA lands (distinct sizes
            # so no two warm-up matmuls are identical instructions).
            pwarm = wpool.tile([128, 512], f32)
            for i in range(N_WARM):
                nc.tensor.matmul(
                    pwarm[:, 0 : 512 - i],
                    scr[:, 0:128],
                    scr[:, 0 : 512 - i],
                    start=True,
                    stop=True,
                )

            def load_piece(b, r0, r1, eng):
                px = xpool.tile([C_IN, P_ROWS, W], bf16, tag="x")
                eng.dma_start(px[:, 0 : r1 - r0, :], x_d[b, :, r0:r1, :])
                return px

            for b in range(B_LOC):
                if b == 0:
                    # chunk0 reads the fused tile; chunk1's piece must land
                    # ~4us after stream start, so it rides Sync right behind
                    # the fused DMA. The rest prefetch on Scalar.
                    pa1 = load_piece(0, 8, 18, nc.sync)   # chunk1 only: slim,
                    pa2 = load_piece(0, 16, 34, nc.sync)  # lands ~0.3us earlier
                    pb = load_piece(0, 24, 42, nc.scalar)
                    pc = load_piece(0, 40, 58, nc.scalar)
                    pd = load_piece(0, 56, 64, nc.scalar)
                    chunk_map = [
                        (x0v, 0), (pa1, 0), (pa2, 0), (pb, 0),
                        (pb, 8), (pc, 0), (pc, 8), (pd, 0),
                    ]
                else:
                    piece_tiles = [
                        load_piece(b, 2 * RPC * k, min(2 * RPC * k + P_ROWS, H),
                                   nc.scalar)
                        for k in range(N_PIECES)
                    ]
                    chunk_map = [
                        (piece_tiles[c // 2], (c % 2) * RPC)
                        for c in range(N_CHUNKS)
                    ]
                def do_group(px, li, b, half, i0, r, store_eng=nc.sync):
                    ps = ppool.tile([128, RPC, W_OUT], f32, tag="ps")
                    for uv in range(KH * KW):
                        u, v = divmod(uv, KW)
                        nc.tensor.matmul(
                            ps[:, 0:r, :],
                            lhsT(half, uv),
                            px[:, li + u : li + u + r, v : v + W_OUT],
                            start=(uv == 0),
                            stop=(uv == KH * KW - 1),
                        )
                    o_t = opool.tile([128, RPC, W_OUT], bf16, tag="o")
                    nc.vector.tensor_scalar_add(
                        o_t[:, 0:r, :], ps[:, 0:r, :], b_t[:, half : half + 1]
                    )
                    store_eng.dma_start(
                        y_d[b, half, :, i0 : i0 + r, :], o_t[:, 0:r, :]
                    )

                for chunk in range(N_CHUNKS):
                    i0 = chunk * RPC
                    r = min(RPC, H_OUT - i0)
                    px, li = chunk_map[chunk]
                    for half in range(N_HALF):
                        last = (b == B_LOC - 1 and chunk == N_CHUNKS - 1
                                and half == N_HALF - 1)
                        if last:
                            # Split the very last PSUM group so the first
                            # sub-group's evac + store hide under the second
                            # sub-group's matmuls, shrinking the kernel tail.
                            h1 = r - 2
                            # Sub-A's store rides Scalar so sub-B's final
                            # store isn't queued behind it on the Sync seq.
                            do_group(px, li, b, half, i0, h1,
                                     store_eng=nc.scalar)
                            do_group(px, li + h1, b, half, i0 + h1, r - h1)
                        else:
                            do_group(px, li, b, half, i0, r)

    nc.compile()
    if not nc.is_finalized():
        nc.finalize()
    return nc


def kernel(inputs, weights, bias, profile=False, trace_kwargs=None):
    x_b = np.ascontiguousarray(
        np.asarray(inputs, dtype=np.float32).astype(ml_dtypes.bfloat16)
    )
    # [O, C, KH, KW] -> [C, half, KH*KW, o_local]  (lhsT layout: contraction dim
    # on partitions; each half contiguous per partition for fast DMA)
    w_t = np.ascontiguousarray(
        np.asarray(weights, dtype=np.float32)
        .reshape(N_HALF, 128, C_IN, KH * KW)
        .transpose(2, 0, 3, 1)
        .astype(ml_dtypes.bfloat16)
    )
    w_flat = w_t.reshape(C_IN, W_LEN)
    wf = np.ascontiguousarray(
        np.concatenate(
            [w_t[:, 0].reshape(C_IN, -1), w_t[:, 1, H1A:].reshape(C_IN, -1)],
            axis=1,
        )
    )
    w1a_arr = np.ascontiguousarray(w_t[:, 1, 0:H1A])
    # [C_OUT, 1] -> [128, N_HALF] with bias_sb[p, h] = bias[h*128 + p]
    b_t = np.ascontiguousarray(
        np.asarray(bias, dtype=np.float32).reshape(N_HALF, 128).T
    )

    if "nc" not in _cached:
        _cached["nc"] = _build_nc()
    nc = _cached["nc"]

    in_maps = []
    for i in range(N_CORES):
        shard = x_b[i * B_LOC : (i + 1) * B_LOC]
        xw0 = np.ascontiguousarray(
            np.concatenate(
                [shard[0, :, 0:X0_ROWS, :].reshape(C_IN, X0_LEN), wf], axis=1
            )
        )
        in_maps.append({"xw0": xw0, "w1a": w1a_arr, "x": shard, "bias_in": b_t})
    res = run_bass_kernel_spmd(
        nc,
        in_maps,
        list(range(N_CORES)),
        trace=profile,
        **(trace_kwargs or {}),
    )
    _cached["last_result"] = res

    shards = []
    for i in range(N_CORES):
        y = res.results[i]["y"]  # [B_LOC, 2, 128, 62, 62] bf16
        shards.append(
            np.asarray(y).astype(np.float32).reshape(B_LOC, C_OUT, H_OUT, W_OUT)
        )
    return np.ascontiguousarray(np.concatenate(shards, axis=0), dtype=np.float32)



# revision 5
# speedup vs baseline: 1.2946x; 1.2946x over previous
"""Trainium2 Bass kernel for 3x3 VALID conv: x[32,128,64,64] * w[256,128,3,3] + bias.

Strategy (v2 — 1D Winograd F(2,3) along H):
  - Data-parallel over batch: 8 cores x 4 images; weights/bias replicated.
  - Winograd F(2,3) on the H axis cuts PE matmul columns by 1/3 vs direct:
    for each row-tile it (2 output rows), 4 transformed inputs
        t0 = x[2it]   - x[2it+2]
        t1 = x[2it+1] + x[2it+2]
        t2 = x[2it+2] - x[2it+1]
        t3 = x[2it+1] - x[2it+3]
    feed 4 point-GEMMs (contraction C_IN=128 on partitions, W-taps v=0..2
    accumulated in PSUM):  M_p[o, it, j] = sum_v sum_c gw[p,v][c,o] t_p[c,it,j+v]
    with gw = G @ w along u (host-side, fp32 then bf16).
    Outputs:  y[2it]   = m0 + m1 + m2 + bias
              y[2it+1] = m1 - m2 - m3 + bias
  - Engine split per (chunk of 8 row-tiles, half of C_OUT):
      PE:  12 matmuls of N=ct*62 cols (4 points x 3 taps)
      ACT: one Copy activation evacuates all 4 M points PSUM->SBUF bf16
      DVE: input transform (4 tensor_tensor) + 4-op output combine with the
           bias folded via scalar_tensor_tensor: y0=(m0+b)+m1+m2,
           y1=((m1+b)-m2)-m3
  - PSUM: 2 groups x [128,4,512] fp32 = 8 banks exactly; (chunk,half) groups
    alternate so evacuation overlaps the next group's matmuls.
  - bf16 throughout (accuracy ~5e-3 << 2e-2 tol); PSUM accumulates fp32.
  - Critical-path fusion kept from v1: host packs [x img0 rows 0-17 |
    half0 transformed taps] into one DRAM tensor, loaded by ONE DMA as the
    Sync ring's first batch; half1 taps ride Sync right behind. Dummy
    matmuls on a zeroed tile ramp the PE clock while that DMA lands; a
    dummy activation preloads the ACT table set.
  - DMA rings: Sync = fused xw + w half1 + img0 pieces 1,2 + half0 stores;
    GpSimd = bias + remaining pieces + half1 stores (ACT queue stays free
    for evacuation compute).
"""

import numpy as np
import ml_dtypes

import concourse.bacc as bacc
import concourse.tile as tile
from concourse import mybir
from concourse.bass_utils import run_bass_kernel_spmd

N_CORES = 8
B_FULL, C_IN, H, W = 32, 128, 64, 64
C_OUT, KH, KW = 256, 3, 3
B_LOC = B_FULL // N_CORES          # images per core
H_OUT = W_OUT = H - KH + 1         # 62
N_HALF = C_OUT // 128              # 2 output-channel halves
NT = H_OUT // 2                    # 31 row-tiles (2 output rows each)
CHUNKS = [(0, 8), (8, 8), (16, 8), (24, 7)]   # (first tile, tiles in chunk)
N_PTS = 4                          # F(2,3) points
N_WARM = 8                         # DVFS warm-up matmuls
X0_ROWS = 18                       # x img0 rows 0..17 ride the fused tile
X0_LEN = X0_ROWS * W               # 1152
GW_LEN = N_PTS * KW * 128          # 1536 per half
XW_LEN = X0_LEN + GW_LEN           # fused tile: x piece0 + half0 taps

_cached = {}


def _build_nc():
    f32 = mybir.dt.float32
    bf16 = mybir.dt.bfloat16
    AF = mybir.ActivationFunctionType
    ALU = mybir.AluOpType
    nc = bacc.Bacc()

    xw_d = nc.declare_dram_parameter("xw0", [C_IN, XW_LEN], bf16, isOutput=False)
    w1_d = nc.declare_dram_parameter("w1", [C_IN, GW_LEN], bf16, isOutput=False)
    x_d = nc.declare_dram_parameter("x", [B_LOC, C_IN, H, W], bf16, isOutput=False)
    b_d = nc.declare_dram_parameter("bias_in", [128, N_HALF], f32, isOutput=False)
    y_d = nc.declare_dram_parameter(
        "y", [B_LOC, N_HALF, 128, H_OUT, W_OUT], bf16, isOutput=True
    )

    with tile.TileContext(nc) as tc:
        with (
            tc.tile_pool(name="const", bufs=1) as cpool,
            tc.tile_pool(name="xin", bufs=5) as xpool,
            tc.tile_pool(name="tin", bufs=8) as tpool,
            tc.tile_pool(name="mev", bufs=3) as mpool,
            tc.tile_pool(name="yout", bufs=4) as ypool,
            tc.tile_pool(name="tmp", bufs=4) as spool,
            tc.tile_pool(name="psum", bufs=2, space="PSUM") as ppool,
        ):
            ct = cpool.tile([C_IN, XW_LEN], bf16)
            w1_t = cpool.tile([C_IN, GW_LEN], bf16)
            b_t = cpool.tile([128, N_HALF], f32)
            scr = cpool.tile([128, 512], bf16)
            actw = cpool.tile([128, 8], bf16)

            nc.vector.memset(scr[:], 0.0)

            # Critical-path DMAs: fused (x piece0 + half0 taps) first on Sync,
            # half1 taps right behind; bias leads the GpSimd ring.
            nc.sync.dma_start(ct[:], xw_d[:])
            nc.sync.dma_start(w1_t[:], w1_d[:])
            nc.gpsimd.dma_start(b_t[:], b_d[:])

            # Preload the ACT function-table set off the critical path.
            nc.scalar.activation(actw[:], scr[:, 0:8], AF.Copy)

            x0v = ct[:, 0:X0_LEN].rearrange("c (h w) -> c h w", w=W)
            gw0 = ct[:, X0_LEN:XW_LEN].rearrange("c (t o) -> c t o", t=N_PTS * KW)
            gw1 = w1_t.rearrange("c (t o) -> c t o", t=N_PTS * KW)

            def lhsT(half, p, v):
                gv = gw0 if half == 0 else gw1
                return gv[:, p * KW + v, :]

            # Ramp the PE p-state while the fused DMA lands. Warm-ups write
            # the first PSUM group buffer (its first real user is the 3rd
            # group, long after these complete). Distinct sizes so no two
            # warm-up matmuls are identical instructions.
            pwarm = ppool.tile([128, N_PTS, 512], f32, tag="ps")
            for i in range(N_WARM):
                nc.tensor.matmul(
                    pwarm[:, i % N_PTS, 0 : 512 - i],
                    scr[:, 0:128],
                    scr[:, 0 : 512 - i],
                    start=True,
                    stop=True,
                )

            def load_piece(b, c, eng):
                r0 = 16 * c
                r1 = min(r0 + 18, H)
                px = xpool.tile([C_IN, X0_ROWS, W], bf16, tag="x")
                eng.dma_start(px[:, 0 : r1 - r0, :], x_d[b, :, r0:r1, :])
                return px

            # Per-(b,chunk) input pieces; img0 piece0 comes from the fused tile.
            piece = {}
            piece[(0, 0)] = x0v
            piece[(0, 1)] = load_piece(0, 1, nc.sync)
            piece[(0, 2)] = load_piece(0, 2, nc.sync)
            piece[(0, 3)] = load_piece(0, 3, nc.gpsimd)

            def transform(b, c):
                # DVE input transform for one chunk: 4 point-tiles.
                if (b, c) not in piece:
                    piece[(b, c)] = load_piece(b, c, nc.gpsimd)
                px = piece[(b, c)]
                ctn = CHUNKS[c][1]
                ts = [
                    tpool.tile([C_IN, 8, W], bf16, tag="t", name=f"t{p}")
                    for p in range(N_PTS)
                ]
                r = lambda a: px[:, a : a + 2 * ctn - 1 : 2, :]
                nc.vector.tensor_tensor(ts[0][:, 0:ctn, :], r(0), r(2), ALU.subtract)
                nc.vector.tensor_tensor(ts[1][:, 0:ctn, :], r(1), r(2), ALU.add)
                nc.vector.tensor_tensor(ts[2][:, 0:ctn, :], r(2), r(1), ALU.subtract)
                nc.vector.tensor_tensor(ts[3][:, 0:ctn, :], r(1), r(3), ALU.subtract)
                return ts

            def do_group(b, c, half, ts, store_eng):
                it0, ctn = CHUNKS[c]
                n = ctn * W_OUT
                ps = ppool.tile([128, N_PTS, 512], f32, tag="ps")
                for p in range(N_PTS):
                    for v in range(KW):
                        nc.tensor.matmul(
                            ps[:, p, 0:n],
                            lhsT(half, p, v),
                            ts[p][:, 0:ctn, v : v + W_OUT],
                            start=(v == 0),
                            stop=(v == KW - 1),
                        )
                # Evacuate all 4 points in one ACT instruction (bf16 cast).
                m = mpool.tile([128, N_PTS, 8 * W_OUT], bf16, tag="m")
                nc.scalar.activation(m[:, :, 0:n], ps[:, :, 0:n], AF.Copy)
                mv = [
                    m[:, p, 0:n].rearrange("c (h w) -> c h w", w=W_OUT)
                    for p in range(N_PTS)
                ]
                bh = b_t[:, half : half + 1]
                yt = ypool.tile([128, 16, W_OUT], bf16, tag="y")
                y0 = yt[:, 0 : 2 * ctn : 2, :]
                y1 = yt[:, 1 : 2 * ctn : 2, :]
                s = spool.tile([128, 8, W_OUT], bf16, tag="s")
                sv = s[:, 0:ctn, :]
                nc.vector.scalar_tensor_tensor(
                    sv, mv[0], bh, mv[1], ALU.add, ALU.add
                )
                nc.vector.tensor_tensor(y0, sv, mv[2], ALU.add)
                u = spool.tile([128, 8, W_OUT], bf16, tag="s")
                uv = u[:, 0:ctn, :]
                nc.vector.scalar_tensor_tensor(
                    uv, mv[1], bh, mv[2], ALU.add, ALU.subtract
                )
                nc.vector.tensor_tensor(y1, uv, mv[3], ALU.subtract)
                store_eng.dma_start(
                    y_d[b, half, :, 2 * it0 : 2 * (it0 + ctn), :],
                    yt[:, 0 : 2 * ctn, :],
                )

            order = [(b, c) for b in range(B_LOC) for c in range(len(CHUNKS))]
            ts_cur = transform(0, 0)
            for gi, (b, c) in enumerate(order):
                ts_next = None
                if gi + 1 < len(order):
                    ts_next = transform(*order[gi + 1])
                do_group(b, c, 0, ts_cur, nc.sync)
                do_group(b, c, 1, ts_cur, nc.gpsimd)
                ts_cur = ts_next

    nc.compile()
    if not nc.is_finalized():
        nc.finalize()
    return nc


_G = np.array(
    [[1.0, 0.0, 0.0], [0.5, 0.5, 0.5], [0.5, -0.5, 0.5], [0.0, 0.0, 1.0]],
    dtype=np.float32,
)


def kernel(inputs, weights, bias, profile=False, trace_kwargs=None):
    x_b = np.ascontiguousarray(
        np.asarray(inputs, dtype=np.float32).astype(ml_dtypes.bfloat16)
    )
    # gw[p,v][c,o]: Winograd-transformed taps, [c, half, p*3+v, o_local]
    w = np.asarray(weights, dtype=np.float32)
    gw = np.einsum("pu,ocuv->cpvo", _G, w)          # [128, 4, 3, 256]
    gwh = (
        gw.reshape(C_IN, N_PTS * KW, N_HALF, 128)
        .transpose(0, 2, 1, 3)
        .astype(ml_dtypes.bfloat16)
    )                                                # [128, 2, 12, 128]
    w0_flat = np.ascontiguousarray(gwh[:, 0].reshape(C_IN, GW_LEN))
    w1_flat = np.ascontiguousarray(gwh[:, 1].reshape(C_IN, GW_LEN))
    # [C_OUT, 1] -> [128, N_HALF] with b_t[p, h] = bias[h*128 + p]
    b_t = np.ascontiguousarray(
        np.asarray(bias, dtype=np.float32).reshape(N_HALF, 128).T
    )

    if "nc" not in _cached:
        _cached["nc"] = _build_nc()
    nc = _cached["nc"]

    in_maps = []
    for i in range(N_CORES):
        shard = x_b[i * B_LOC : (i + 1) * B_LOC]
        xw0 = np.ascontiguousarray(
            np.concatenate(
                [shard[0, :, 0:X0_ROWS, :].reshape(C_IN, X0_LEN), w0_flat], axis=1
            )
        )
        in_maps.append(
            {"xw0": xw0, "w1": w1_flat, "x": shard, "bias_in": b_t}
        )
    res = run_bass_kernel_spmd(
        nc,
        in_maps,
        list(range(N_CORES)),
        trace=profile,
        **(trace_kwargs or {}),
    )
    _cached["last_result"] = res

    shards = []
    for i in range(N_CORES):
        y = res.results[i]["y"]  # [B_LOC, 2, 128, 62, 62] bf16
        shards.append(
            np.asarray(y).astype(np.float32).reshape(B_LOC, C_OUT, H_OUT, W_OUT)
        )
    return np.ascontiguousarray(np.concatenate(shards, axis=0), dtype=np.float32)


# revision 7
# speedup vs baseline: 1.3169x; 1.0172x over previous
"""Trainium2 Bass kernel for 3x3 VALID conv: x[32,128,64,64] * w[256,128,3,3] + bias.

Strategy (v3 — 1D Winograd F(2,3) along H):
  - Data-parallel over batch: 8 cores x 4 images; weights/bias replicated.
  - Winograd F(2,3) on the H axis cuts PE matmul columns by 1/3 vs direct:
    for each row-tile it (2 output rows), 4 transformed inputs
        t0 = x[2it]   - x[2it+2]
        t1 = x[2it+1] + x[2it+2]
        t2 = x[2it+2] - x[2it+1]
        t3 = x[2it+1] - x[2it+3]
    feed 4 point-GEMMs (contraction C_IN=128 on partitions, W-taps v=0..2
    accumulated in PSUM):  M_p[o, it, j] = sum_v sum_c gw[p,v][c,o] t_p[c,it,j+v]
    with gw = G @ w along u (host-side, fp32 then bf16).
    Outputs:  y[2it]   = m0 + (m1+bias) + m2
              y[2it+1] = (m1+bias) - m2 - m3
  - Engine split per (chunk of 8 row-tiles, half of C_OUT):
      PE:  12 matmuls of N=ct*62 cols (4 points x 3 taps)
      ACT: two Copy activations evacuate PSUM->SBUF bf16 — {M1 with the
           per-partition bias} and {M0,M2,M3} (PSUM point order m1,m0,m2,m3)
      DVE: input transform (4 tensor_tensor) + 4 plain tensor_tensor
           combines (scalar_tensor_tensor has no 2x uop — avoid it)
  - PSUM: 2 groups x [128,4,512] fp32 = 8 banks exactly; (chunk,half) groups
    alternate so evacuation overlaps the next group's matmuls.
  - bf16 throughout (accuracy ~7e-3 << 2e-2 tol); PSUM accumulates fp32.
  - Startup: x piece0 rides Sync's first batch, half0 taps ride the Scalar
    (ACT) ring's first batch — two parallel HWDGE rings halve the landing
    time; dummy matmuls ramp the PE clock until then, and a dummy
    activation preloads the ACT table set.
  - Tail: the very last (chunk,half) group is sub-split along tile-rows so
    the final evac+combine+store chain is short.
  - DMA rings: Sync = x0 + w1 + img0 pieces 1,2 + half0 stores; GpSimd =
    bias + remaining pieces + half1 stores (ACT queue carries only the w0
    load then pure evacuation compute).
"""

import numpy as np
import ml_dtypes

import concourse.bacc as bacc
import concourse.tile as tile
from concourse import mybir
from concourse.bass_utils import run_bass_kernel_spmd

N_CORES = 8
B_FULL, C_IN, H, W = 32, 128, 64, 64
C_OUT, KH, KW = 256, 3, 3
B_LOC = B_FULL // N_CORES          # images per core
H_OUT = W_OUT = H - KH + 1         # 62
N_HALF = C_OUT // 128              # 2 output-channel halves
NT = H_OUT // 2                    # 31 row-tiles (2 output rows each)
CHUNKS = [(0, 8), (8, 8), (16, 8), (24, 7)]   # (first tile, tiles in chunk)
N_PTS = 4                          # F(2,3) points
SLOT = {1: 0, 0: 1, 2: 2, 3: 3}    # PSUM/evac point order: m1, m0, m2, m3
N_WARM = 10                        # DVFS warm-up matmuls
X0_ROWS = 18
X0_LEN = X0_ROWS * W               # 1152
GW_LEN = N_PTS * KW * 128          # 1536 per half

_cached = {}


def _build_nc():
    f32 = mybir.dt.float32
    bf16 = mybir.dt.bfloat16
    AF = mybir.ActivationFunctionType
    ALU = mybir.AluOpType
    nc = bacc.Bacc()

    x0_d = nc.declare_dram_parameter("x0", [C_IN, X0_LEN], bf16, isOutput=False)
    w0_d = nc.declare_dram_parameter("w0", [C_IN, GW_LEN], bf16, isOutput=False)
    w1_d = nc.declare_dram_parameter("w1", [C_IN, GW_LEN], bf16, isOutput=False)
    x_d = nc.declare_dram_parameter("x", [B_LOC, C_IN, H, W], bf16, isOutput=False)
    b_d = nc.declare_dram_parameter("bias_in", [128, N_HALF], f32, isOutput=False)
    y_d = nc.declare_dram_parameter(
        "y", [B_LOC, N_HALF, 128, H_OUT, W_OUT], bf16, isOutput=True
    )

    with tile.TileContext(nc) as tc:
        with (
            tc.tile_pool(name="const", bufs=1) as cpool,
            tc.tile_pool(name="xin", bufs=5) as xpool,
            tc.tile_pool(name="tin", bufs=8) as tpool,
            tc.tile_pool(name="mev", bufs=3) as mpool,
            tc.tile_pool(name="yout", bufs=4) as ypool,
            tc.tile_pool(name="tmp", bufs=4) as spool,
            tc.tile_pool(name="psum", bufs=2, space="PSUM") as ppool,
        ):
            x0_t = cpool.tile([C_IN, X0_ROWS, W], bf16)
            w0_t = cpool.tile([C_IN, GW_LEN], bf16)
            w1_t = cpool.tile([C_IN, GW_LEN], bf16)
            b_t = cpool.tile([128, N_HALF], f32)
            scr = cpool.tile([128, 512], bf16)
            actw = cpool.tile([128, 8], bf16)

            nc.vector.memset(scr[:], 0.0)

            # Startup DMAs: x piece0 and half0 taps land in parallel on the
            # two HWDGE rings; half1 taps right behind; bias leads GpSimd.
            nc.sync.dma_start(x0_t[:], x0_d.rearrange("c (h w) -> c h w", w=W))
            nc.scalar.dma_start(w0_t[:], w0_d[:])
            nc.sync.dma_start(w1_t[:], w1_d[:])
            nc.gpsimd.dma_start(b_t[:], b_d[:])

            # Preload the ACT function-table set off the critical path.
            nc.scalar.activation(actw[:], scr[:, 0:8], AF.Copy)

            gw0 = w0_t.rearrange("c (t o) -> c t o", t=N_PTS * KW)
            gw1 = w1_t.rearrange("c (t o) -> c t o", t=N_PTS * KW)

            def lhsT(half, p, v):
                gv = gw0 if half == 0 else gw1
                return gv[:, p * KW + v, :]

            # Ramp the PE p-state while the startup DMAs land. Warm-ups
            # write the first PSUM group buffer (its first real user is the
            # 3rd group, long after these complete). Distinct sizes so no
            # two warm-up matmuls are identical instructions.
            pwarm = ppool.tile([128, N_PTS, 512], f32, tag="ps")
            for i in range(N_WARM):
                nc.tensor.matmul(
                    pwarm[:, i % N_PTS, 0 : 512 - i],
                    scr[:, 0:128],
                    scr[:, 0 : 512 - i],
                    start=True,
                    stop=True,
                )

            piece = {}

            def load_piece(b, c, eng):
                r0 = 16 * c
                r1 = min(r0 + 18, H)
                px = xpool.tile([C_IN, X0_ROWS, W], bf16, tag="x")
                eng.dma_start(px[:, 0 : r1 - r0, :], x_d[b, :, r0:r1, :])
                piece[(b, c)] = px

            piece[(0, 0)] = x0_t
            load_piece(0, 1, nc.sync)
            load_piece(0, 2, nc.sync)
            load_piece(0, 3, nc.gpsimd)

            def transform(b, c):
                # DVE input transform for one chunk: 4 point-tiles.
                px = piece[(b, c)]
                ctn = CHUNKS[c][1]
                ts = [
                    tpool.tile([C_IN, 8, W], bf16, tag="t", name=f"t{p}")
                    for p in range(N_PTS)
                ]
                r = lambda a: px[:, a : a + 2 * ctn - 1 : 2, :]
                nc.vector.tensor_tensor(ts[0][:, 0:ctn, :], r(0), r(2), ALU.subtract)
                nc.vector.tensor_tensor(ts[1][:, 0:ctn, :], r(1), r(2), ALU.add)
                nc.vector.tensor_tensor(ts[2][:, 0:ctn, :], r(2), r(1), ALU.subtract)
                nc.vector.tensor_tensor(ts[3][:, 0:ctn, :], r(1), r(3), ALU.subtract)
                return ts

            def do_group(b, c, half, ts, store_eng, subsplit=None):
                it0, ctn = CHUNKS[c]
                ps = ppool.tile([128, N_PTS, 512], f32, tag="ps")
                m = mpool.tile([128, N_PTS, 8 * W_OUT], bf16, tag="m")
                yt = ypool.tile([128, 16, W_OUT], bf16, tag="y")
                bh = b_t[:, half : half + 1]
                for r0, rn in subsplit or [(0, ctn)]:
                    n0, n1 = r0 * W_OUT, (r0 + rn) * W_OUT
                    for p in range(N_PTS):
                        for v in range(KW):
                            nc.tensor.matmul(
                                ps[:, SLOT[p], n0:n1],
                                lhsT(half, p, v),
                                ts[p][:, r0 : r0 + rn, v : v + W_OUT],
                                start=(v == 0),
                                stop=(v == KW - 1),
                            )
                    # Evacuate PSUM->SBUF bf16: {m1 + bias}, then {m0,m2,m3}.
                    nc.scalar.activation(
                        m[:, 0, n0:n1], ps[:, 0, n0:n1], AF.Identity, bias=bh
                    )
                    nc.scalar.activation(m[:, 1:4, n0:n1], ps[:, 1:4, n0:n1], AF.Copy)
                    m3d = [
                        m[:, SLOT[p], n0:n1].rearrange("c (h w) -> c h w", w=W_OUT)
                        for p in range(N_PTS)
                    ]
                    y0 = yt[:, 2 * r0 : 2 * (r0 + rn) : 2, :]
                    y1 = yt[:, 2 * r0 + 1 : 2 * (r0 + rn) : 2, :]
                    s = spool.tile([128, 8, W_OUT], bf16, tag="s")
                    sv = s[:, 0:rn, :]
                    nc.vector.tensor_tensor(sv, m3d[0], m3d[1], ALU.add)
                    nc.vector.tensor_tensor(y0, sv, m3d[2], ALU.add)
                    u = spool.tile([128, 8, W_OUT], bf16, tag="s")
                    uv = u[:, 0:rn, :]
                    nc.vector.tensor_tensor(uv, m3d[1], m3d[2], ALU.subtract)
                    nc.vector.tensor_tensor(y1, uv, m3d[3], ALU.subtract)
                    store_eng.dma_start(
                        y_d[b, half, :, 2 * (it0 + r0) : 2 * (it0 + r0 + rn), :],
                        yt[:, 2 * r0 : 2 * (r0 + rn), :],
                    )

            order = [(b, c) for b in range(B_LOC) for c in range(len(CHUNKS))]
            ts_cur = transform(0, 0)
            for gi, (b, c) in enumerate(order):
                # Prefetch the piece two chunks ahead (lazy, gpsimd ring).
                if gi + 2 < len(order) and order[gi + 2] not in piece:
                    load_piece(*order[gi + 2], nc.gpsimd)
                ts_next = transform(*order[gi + 1]) if gi + 1 < len(order) else None
                last = gi == len(order) - 1
                do_group(b, c, 0, ts_cur, nc.sync)
                do_group(
                    b, c, 1, ts_cur, nc.gpsimd,
                    subsplit=[(0, 4), (4, 3)] if last else None,
                )
                ts_cur = ts_next

    nc.compile()
    if not nc.is_finalized():
        nc.finalize()
    return nc


_G = np.array(
    [[1.0, 0.0, 0.0], [0.5, 0.5, 0.5], [0.5, -0.5, 0.5], [0.0, 0.0, 1.0]],
    dtype=np.float32,
)


def kernel(inputs, weights, bias, profile=False, trace_kwargs=None):
    x_b = np.ascontiguousarray(
        np.asarray(inputs, dtype=np.float32).astype(ml_dtypes.bfloat16)
    )
    # gw[p,v][c,o]: Winograd-transformed taps, [c, half, p*3+v, o_local]
    w = np.asarray(weights, dtype=np.float32)
    gw = np.einsum("pu,ocuv->cpvo", _G, w)          # [128, 4, 3, 256]
    gwh = (
        gw.reshape(C_IN, N_PTS * KW, N_HALF, 128)
        .transpose(0, 2, 1, 3)
        .astype(ml_dtypes.bfloat16)
    )                                                # [128, 2, 12, 128]
    w0_flat = np.ascontiguousarray(gwh[:, 0].reshape(C_IN, GW_LEN))
    w1_flat = np.ascontiguousarray(gwh[:, 1].reshape(C_IN, GW_LEN))
    # [C_OUT, 1] -> [128, N_HALF] with b_t[p, h] = bias[h*128 + p]
    b_t = np.ascontiguousarray(
        np.asarray(bias, dtype=np.float32).reshape(N_HALF, 128).T
    )

    if "nc" not in _cached:
        _cached["nc"] = _build_nc()
    nc = _cached["nc"]

    in_maps = []
    for i in range(N_CORES):
        shard = x_b[i * B_LOC : (i + 1) * B_LOC]
        x0 = np.ascontiguousarray(shard[0, :, 0:X0_ROWS, :].reshape(C_IN, X0_LEN))
        in_maps.append(
            {"x0": x0, "w0": w0_flat, "w1": w1_flat, "x": shard, "bias_in": b_t}
        )
    res = run_bass_kernel_spmd(
        nc,
        in_maps,
        list(range(N_CORES)),
        trace=profile,
        **(trace_kwargs or {}),
    )
    _cached["last_result"] = res

    shards = []
    for i in range(N_CORES):
        y = res.results[i]["y"]  # [B_LOC, 2, 128, 62, 62] bf16
        shards.append(
            np.asarray(y).astype(np.float32).reshape(B_LOC, C_OUT, H_OUT, W_OUT)
        )
    return np.ascontiguousarray(np.concatenate(shards, axis=0), dtype=np.float32)
